# revision 33
# baseline (speedup 1.0000x reference)
"""Trainium2 Bass kernel for nn_ContextEncoderLayer (per-position cross-attention
encoder layer).  Shards the sequence dim L across 8 NeuronCores; each core runs an
identical Bass/Tile program on its 256-position shard.

Layout strategy (per core, L_C=256 positions):
  - attention works on (l,d)-row tiles: 64 tiles of [128 partitions = 4 positions x 32
    candidates, free = (h,dh) = 1024].
  - LOW-RANK K PATH: instead of projecting K for all 8192 rows, compute
    qT = (src@Wq+bq)^T (phase Q), then qWT = qT @ Wk^T per head (phase W, fp8,
    32x scale) and get scores directly as one fused PE pass per tile:
    scores[ld, (h,l')] = x_tile^T-chunk-pairs (stationary, shared with the V
    projection) x qWT pair-slices (fp8 DoubleRow).  Off-diagonal (h,l') slots
    are masked with a -30000 constant before exp; attn_mask rides the ACT Exp
    per-partition bias; a 4-slot DVE reduce yields ex[ld, h].
  - V projection: fp8 DoubleRow PE matmuls, x^T chunk-pairs stationary, fp8 Wv
    (8x host prescale) streamed; bv folded into the residual (src + bv).
  - candidate sum + softmax denominator: fp8 DoubleRow selector matmuls over
    tile PAIRS (interleaved pv/ex streams, shifted 0/1 and 0/8 selector banks).
  - LN1/LN2: var = E[x^2]-mean^2 with DVE reduce || ACT Square-accum in
    parallel, fused (r-mean)*rstd tensor_scalar; g1/beta1 applied inside the
    x^T transpose evictions (per-partition ACT scale/bias APs).
  - FFN: W1/W2 bf16 (fp8 FFN fails the 2e-2 gate), W2 resident + W1 prefetched
    during phase B, GELU+bias fused into the PSUM->SBUF eviction; output fp32.
"""

import sys

sys.path.insert(0, "/opt/trn_rl_repo")

from contextlib import ExitStack

import numpy as np
import ml_dtypes

import concourse.bacc as bacc
import concourse.tile as tile
from concourse import mybir
from concourse.bass_utils import run_bass_kernel_spmd
from concourse.masks import make_identity

L, D, DM, H, FF = 2048, 32, 1024, 16, 4096
DH = DM // H  # 64
SCALE = float(np.sqrt(DH))  # 8.0
NCORES = 8
LC = L // NCORES  # 256 positions per core
NT = LC * D // 128  # 64 (l,d)-row tiles per core
NLT = LC // 128  # 2 l-tiles per core
NC_DM = DM // 128  # 8 dm chunks
CH = NC_DM + 1  # 9: 8 chunks + ones-row (bias fold, Q path only)
NFF = FF // 128  # 32 ff chunks
TPB = NT // NLT  # 32 (l,d)-tiles per l-tile
BF = mybir.dt.bfloat16
F32 = mybir.dt.float32
F8 = mybir.dt.float8e4
F8NP = ml_dtypes.float8_e4m3
FP8_SCALE = 8.0  # host pre-scale on Wk/Wv so fp8 weights use the normal range
QT_SCALE = 4.0  # q carried at 4x (bf16) into the qW GEMM
QW_NET = FP8_SCALE * QT_SCALE  # qWT is stored at 32x true qW

_CACHE = {}
PENDING_DELAY = True


def _sel_matrices(value=1.0):
    """Shifted selector-pair bank [128, 2, 256] (256 stride: dual-fp8 LDW
    requires power-of-2 chunk strides).

    Slice [:, :, 120-8*jp : 248-8*jp] is the DoubleRow lhsT for tile pair
    (2jp, 2jp+1) of an l-tile: slice[p, two, c] = value iff
    c == 8*jp + 4*two + p//32  (ctx partition 4*tt + p//32 for tile tt)."""
    g = np.zeros((128, 2, 256), dtype=np.float32)
    for p in range(128):
        for two in range(2):
            g[p, two, 120 + 4 * two + p // 32] = value
    return g.reshape(128, 2 * 256).astype(F8NP)


def _negdiag():
    """[128, 64] additive pre-exp mask: 0 on a partition's own l-slot
    (h*4 + p//32), -30000 on the other tiles' slots (garbage scores)."""
    m = np.full((128, 64), -30000.0, dtype=np.float32)
    for p in range(128):
        for h in range(16):
            m[p, h * 4 + p // 32] = 0.0
    return m


def _build_nc(repeat=1):
    nc = bacc.Bacc("TRN2", target_bir_lowering=False, debug=False, num_devices=NCORES)

    # ---------------- I/O ----------------
    xt_in = nc.dram_tensor("xt", [NT, 128, NC_DM * 128], F8, kind="ExternalInput")
    wv_in = nc.dram_tensor("wv", [128, NC_DM * 1024], F8, kind="ExternalInput")
    wq_in = nc.dram_tensor("wq", [128, NC_DM * 1024], F8, kind="ExternalInput")
    st_in = nc.dram_tensor("st", [128, NC_DM * LC], F8, kind="ExternalInput")
    wkt_in = nc.dram_tensor("wkt", [64, H * NC_DM * 128], F8, kind="ExternalInput")
    bq_in = nc.dram_tensor("bqp", [64, H], F32, kind="ExternalInput")
    src_in = nc.dram_tensor("srcr", [LC, DM], F32, kind="ExternalInput")
    mask_in = nc.dram_tensor("maskp", [128, NT], F32, kind="ExternalInput")
    b1_in = nc.dram_tensor("b1p", [128, NFF], F32, kind="ExternalInput")
    bb2_in = nc.dram_tensor("bb2p", [1, DM], BF, kind="ExternalInput")
    g1_in = nc.dram_tensor("g1p", [1, DM], BF, kind="ExternalInput")
    g1t_in = nc.dram_tensor("g1tp", [128, NC_DM], F32, kind="ExternalInput")
    be1t_in = nc.dram_tensor("be1tp", [128, NC_DM], F32, kind="ExternalInput")
    g2_in = nc.dram_tensor("g2p", [1, DM], BF, kind="ExternalInput")
    be2_in = nc.dram_tensor("be2p", [1, DM], BF, kind="ExternalInput")
    w1_in = nc.dram_tensor("w1p", [NFF, 128, 1024], BF, kind="ExternalInput")
    w2_in = nc.dram_tensor("w2p", [128, NFF * 1024], BF, kind="ExternalInput")
    out = nc.dram_tensor("out", [LC, DM], F32, kind="ExternalOutput")

    g_const = nc.inline_tensor(np.asarray(_sel_matrices(1.0)), name="gsel")
    g8_const = nc.inline_tensor(np.asarray(_sel_matrices(FP8_SCALE)), name="g8sel")
    negd_const = nc.inline_tensor(np.asarray(_negdiag()), name="negd")

    AL = mybir.AluOpType
    AF = mybir.ActivationFunctionType
    DR = mybir.MatmulPerfMode.DoubleRow

    with tile.TileContext(nc) as tc, ExitStack() as top:
        consts = top.enter_context(tc.tile_pool(name="consts", bufs=1))

        # ------- constants / params resident in SBUF (Q-phase deps first) -------
        st_sb = consts.tile([128, NC_DM * LC], F8)
        nc.sync.dma_start(st_sb[:], st_in[:])
        wq_sb = consts.tile([128, NC_DM * 1024], F8)
        for _h in range(H):
            nc.sync.dma_start(
                wq_sb[:, _h * 512 : (_h + 1) * 512], wq_in[:, _h * 512 : (_h + 1) * 512]
            )
        wkt_sb = consts.tile([64, H * NC_DM * 128], F8)
        nc.sync.dma_start(wkt_sb[:], wkt_in[:])
        bq_sb = consts.tile([64, H], F32)
        nc.scalar.dma_start(bq_sb[:], bq_in[:])
        negd_sb = consts.tile([128, 64], F32)
        nc.scalar.dma_start(negd_sb[:], negd_const[:])
        wv_sb = consts.tile([128, NC_DM * 1024], F8)
        nc.sync.dma_start(wv_sb[:], wv_in[:])
        qWT_sb = consts.tile([128, NC_DM * NT * 64], F8)
        mask_sb = consts.tile([128, NT], F32)
        nc.scalar.dma_start(mask_sb[:], mask_in[:])
        b1_sb = consts.tile([128, NFF], F32)
        nc.scalar.dma_start(b1_sb[:], b1_in[:])
        g_sb = consts.tile([128, 2 * 256], F8)
        nc.scalar.dma_start(g_sb[:], g_const[:])
        g8_sb = consts.tile([128, 2 * 256], F8)
        nc.scalar.dma_start(g8_sb[:], g8_const[:])
        # big resident W2 tile; its DMA is issued mid-phase-B (ACT ring) so the
        # transfer neither blocks the SP ring nor eats startup DMA bandwidth
        w2_sb = consts.tile([128, NFF * 1024], BF)
        ident = consts.tile([128, 128], BF)
        make_identity(nc, ident[:])
        eps_sb = consts.tile([128, 1], F32)
        nc.vector.memset(eps_sb[:], 1e-5)

        def rep128(name, src):  # [1, DM] -> [128, DM] partition-broadcast, bf16
            t = consts.tile([128, DM], BF, name=name)
            nc.gpsimd.dma_start(t[:], src[0:1, :].broadcast_to([128, DM]))
            return t

        g1_rep = rep128("g1_rep", g1_in)
        bb2_rep = rep128("bb2_rep", bb2_in)
        g2_rep = rep128("g2_rep", g2_in)
        be2_rep = rep128("be2_rep", be2_in)
        g1t_sb = consts.tile([128, NC_DM], F32)
        nc.scalar.dma_start(g1t_sb[:], g1t_in[:])
        be1t_sb = consts.tile([128, NC_DM], F32)
        nc.scalar.dma_start(be1t_sb[:], be1t_in[:])

        xres = top.enter_context(tc.tile_pool(name="xres", bufs=1))
        xtp = top.enter_context(tc.tile_pool(name="xtp", bufs=1))
        w1_pool = top.enter_context(tc.tile_pool(name="w1_pool", bufs=3))

        for _rep in range(repeat):
            x_tiles = []
            w1_early = []
            xT_sb = xtp.tile([128, NC_DM * LC], BF, name=f"xT{_rep}", tag="xT")
            # ------- phase Q: qT[dh, (h,l)] = QT_SCALE*(src @ Wq + bq)^T, bf16 -------
            qtpool = ExitStack()
            qtp = qtpool.enter_context(tc.tile_pool(name="qtp", bufs=1))
            qT_sb = qtp.tile([64, H * LC], BF, name=f"qT{_rep}", tag="qT")
            with tc.tile_pool(name="qps", bufs=2, space="PSUM") as qpsp:
                st3 = st_sb.rearrange("p (c f) -> p c f", f=LC)
                wq4 = wq_sb.rearrange("p (h c d) -> p h c d", c=NC_DM, d=64)
                for h in range(H):
                    qps = qpsp.tile([64, LC], F32, name=f"qps{_rep}_{h}", tag="qps")
                    for pc in range(NC_DM // 2):
                        nc.tensor.matmul(
                            qps[:],
                            wq4[:, h, 2 * pc : 2 * pc + 2, :],
                            st3[:, 2 * pc : 2 * pc + 2, :],
                            start=(pc == 0),
                            stop=(pc == NC_DM // 2 - 1),
                            perf_mode=DR,
                        )
                    nc.scalar.activation(
                        qT_sb[:, h * LC : (h + 1) * LC],
                        qps[:],
                        AF.Identity,
                        bias=bq_sb[:, h : h + 1],
                        scale=QT_SCALE / FP8_SCALE,
                    )
            # ------- phase W: qWT[n, (h, tile, l')] = 32x qW = q @ Wk^T, fp8 -------
            with tc.tile_pool(name="qwps", bufs=3, space="PSUM") as qwpsp:
                qw5 = qWT_sb.rearrange("p (c h t f) -> p c h t f", h=H, t=NT, f=4)
                for h in range(H):
                    for c4 in range(NC_DM // 4):
                        qwps = qwpsp.tile(
                            [128, 4 * LC], F32, name=f"qw{_rep}_{h}_{c4}", tag="qwps"
                        )
                        for cj in range(4):
                            c = 4 * c4 + cj
                            nc.tensor.matmul(
                                qwps[:, cj * LC : (cj + 1) * LC],
                                wkt_sb[
                                    :, (h * NC_DM + c) * 128 : (h * NC_DM + c + 1) * 128
                                ],
                                qT_sb[:, h * LC : (h + 1) * LC],
                                start=True,
                                stop=True,
                            )
                        # one wide strided eviction per 4 chunks, alternating
                        # engines so the PE is not eviction-throttled
                        dst = qw5[:, 4 * c4 : 4 * c4 + 4, h, :, :]
                        if (h * 2 + c4) % 2 == 0:
                            nc.scalar.activation(dst, qwps[:], AF.Copy)
                        else:
                            nc.vector.tensor_copy(dst, qwps[:])
            qtpool.close()

            # ---------------- phase B: projections + attention ----------------
            with ExitStack() as pb:
                proj_ps = pb.enter_context(tc.tile_pool(name="proj_ps", bufs=3, space="PSUM"))
                acc_ps = pb.enter_context(tc.tile_pool(name="acc_ps", bufs=1, space="PSUM"))
                den_psp = pb.enter_context(tc.tile_pool(name="den_ps", bufs=1, space="PSUM"))
                tp_psp = pb.enter_context(tc.tile_pool(name="tp_ps", bufs=1, space="PSUM"))
                sc_psp = pb.enter_context(tc.tile_pool(name="sc_ps", bufs=1, space="PSUM"))
                xt_pool = pb.enter_context(tc.tile_pool(name="xt_pool", bufs=5))
                kb_pool = pb.enter_context(tc.tile_pool(name="kb_pool", bufs=3))
                t_pool = pb.enter_context(tc.tile_pool(name="t_pool", bufs=2))
                pv_pool = pb.enter_context(tc.tile_pool(name="pv_pool", bufs=4))
                sc_pool = pb.enter_context(tc.tile_pool(name="sc_pool", bufs=3))
                ln_pool = pb.enter_context(tc.tile_pool(name="ln_pool", bufs=1))

                def proj_half(ps, w_sb, xt3, h):
                    # fp8 DoubleRow: 4 chunk-pairs for one 512-wide output half
                    w3 = w_sb.rearrange("p (c n) -> p c n", n=1024)
                    for pc in range(NC_DM // 2):
                        nc.tensor.matmul(
                            ps[:],
                            xt3[:, 2 * pc : 2 * pc + 2, :],
                            w3[:, 2 * pc : 2 * pc + 2, h * 512 : (h + 1) * 512],
                            start=(pc == 0),
                            stop=(pc == NC_DM // 2 - 1),
                            perf_mode=DR,
                        )

                pending = []

                g3 = g_sb.rearrange("p (two c) -> p two c", c=256)
                g83 = g8_sb.rearrange("p (two c) -> p two c", c=256)

                def emit_sel(p):
                    # one fp8 DoubleRow matmul per tile PAIR (2jp, 2jp+1): the
                    # two tiles' pv/ex stream interleaved; shifted selector-pair
                    # slice as lhsT maps rows to ctx partitions 4*tt + p//32
                    (pvp, exp2, jp, ctx_ps, den_ps) = p
                    off = 120 - 8 * jp
                    pv3 = pvp.rearrange("p (two f) -> p two f", two=2)
                    for h in range(2):
                        nc.tensor.matmul(
                            ctx_ps[:, h * 512 : (h + 1) * 512],
                            g3[:, :, off : off + 128],
                            pv3[:, :, h * 512 : (h + 1) * 512],
                            start=(jp == 0),
                            stop=(jp == TPB // 2 - 1),
                            perf_mode=DR,
                        )
                    nc.tensor.matmul(
                        den_ps[:],
                        g83[:, :, off : off + 128],
                        exp2.rearrange("p (two f) -> p two f", two=2),
                        start=(jp == 0),
                        stop=(jp == TPB // 2 - 1),
                        perf_mode=DR,
                    )

                qw6 = qWT_sb.rearrange("p (c h t f) -> p c h t f", h=H, t=NT, f=4)
                for lt in range(NLT):
                    ctx_ps = acc_ps.tile([128, 1024], F32, name=f"{_rep}_ctx_ps{lt}", tag="ctx")
                    den_ps = den_psp.tile([128, 16], F32, name=f"{_rep}_den_ps{lt}", tag="den")
                    for tt in range(TPB):
                        t = lt * TPB + tt
                        if lt == 0 and tt >= 3 and tt % 4 == 3:
                            s = tt // 4  # 0..7: stream W2 in 1MB slices between xt loads
                            nc.sync.dma_start(
                                w2_sb[:, s * 4096 : (s + 1) * 4096],
                                w2_in[:, s * 4096 : (s + 1) * 4096],
                            )
                        if lt == 1 and tt >= 26 and tt % 2 == 0:
                            cc_pre = (tt - 26) // 2  # 0..2: prefetch first W1 chunks
                            w1e = w1_pool.tile(
                                [128, 1024], BF, name=f"{_rep}_w1t{cc_pre}", tag="w1t"
                            )
                            nc.sync.dma_start(w1e[:], w1_in[cc_pre])
                            w1_early.append(w1e)
                        xt_sb = xt_pool.tile([128, NC_DM * 128], F8, name=f"{_rep}_xt{t}", tag="xt")
                        nc.sync.dma_start(xt_sb[:], xt_in[t])
                        xt3 = xt_sb.rearrange("p (c m) -> p c m", m=128)
                        # scores[ld, (h,l')] on PE: xt chunk-pairs stationary,
                        # qWT pair-slices streamed (32x true scale)
                        scps = sc_psp.tile([128, 64], F32, name=f"{_rep}_scp{t}", tag="scps")
                        for pc in range(NC_DM // 2):
                            nc.tensor.matmul(
                                scps[:],
                                xt3[:, 2 * pc : 2 * pc + 2, :],
                                qw6[:, 2 * pc : 2 * pc + 2, :, t, :],
                                start=(pc == 0),
                                stop=(pc == NC_DM // 2 - 1),
                                perf_mode=DR,
                            )
                        if PENDING_DELAY and len(pending) >= 2:
                            emit_sel(pending.pop(0))
                        vb = kb_pool.tile([128, 1024], F8, name=f"{_rep}_vb{t}", tag="vb")
                        for hh in range(2):
                            vpsh = proj_ps.tile(
                                [128, 512], F32, name=f"{_rep}_vps{t}_{hh}", tag="proj"
                            )
                            proj_half(vpsh, wv_sb, xt3, hh)
                            if hh == 0:
                                nc.vector.tensor_copy(vb[:, 0:512], vpsh[:])
                            else:
                                nc.scalar.copy(vb[:, 512:1024], vpsh[:])
                        scm = t_pool.tile([128, 64], F32, name=f"{_rep}_scm{t}", tag="t")
                        nc.vector.tensor_tensor(scm[:], scps[:], negd_sb[:], AL.add)
                        ex64 = t_pool.tile([128, 64], BF, name=f"{_rep}_e64{t}", tag="e64")
                        nc.scalar.activation(
                            ex64[:],
                            scm[:],
                            AF.Exp,
                            bias=mask_sb[:, t : t + 1],
                            scale=1.0 / (QW_NET * SCALE),
                        )
                        if tt % 2 == 0:
                            pvp = pv_pool.tile([128, 2048], F8, name=f"{_rep}_pv{t}", tag="pv")
                            exp2 = sc_pool.tile([128, 32], F8, name=f"{_rep}_ex{t}", tag="ex")
                        half = tt % 2
                        with nc.allow_low_precision(reason="4-slot sum, 3 are ~0"):
                            nc.vector.tensor_reduce(
                                exp2[:, half * 16 : half * 16 + 16],
                                ex64.rearrange("p (h x) -> p h x", x=4),
                                axis=mybir.AxisListType.X,
                                op=AL.add,
                            )
                        nc.vector.tensor_tensor(
                            pvp[:, half * 1024 : half * 1024 + 1024].rearrange(
                                "p (h x) -> p h x", x=DH
                            ),
                            vb.rearrange("p (h x) -> p h x", x=DH),
                            exp2[:, half * 16 : half * 16 + 16]
                            .rearrange("p (h o) -> p h o", o=1)
                            .broadcast_to([128, H, DH]),
                            AL.mult,
                        )
                        if tt % 2 == 1:
                            pending.append((pvp, exp2, tt // 2, ctx_ps, den_ps))
                    while pending:
                        emit_sel(pending.pop(0))

                    # ---- l-tile epilogue: softmax-normalize, residual, LN1, x^T ----
                    rd = sc_pool.tile([128, 16], F32, name=f"{_rep}_rd{lt}", tag="rd")
                    nc.vector.reciprocal(rd[:], den_ps[:])
                    ctxn = t_pool.tile([128, 1024], F32, name=f"{_rep}_ctxn{lt}", tag="t")
                    nc.vector.tensor_tensor(
                        ctxn.rearrange("p (h x) -> p h x", x=DH),
                        ctx_ps.rearrange("p (h x) -> p h x", x=DH),
                        rd.rearrange("p (h o) -> p h o", o=1).broadcast_to([128, H, DH]),
                        AL.mult,
                    )
                    src_sb = ln_pool.tile([128, 1024], F32, name=f"{_rep}_srcsb{lt}", tag="srcsb")
                    nc.sync.dma_start(src_sb[:], src_in[lt * 128 : (lt + 1) * 128, :])
                    r = ln_pool.tile([128, 1024], F32, name=f"{_rep}_r{lt}", tag="r")
                    rsum = sc_pool.tile([128, 1], F32, name=f"{_rep}_rsum{lt}", tag="rsum")
                    nc.vector.tensor_tensor(r[:], ctxn[:], src_sb[:], AL.add)
                    # mean/var in parallel on DVE/ACT: var = E[r^2] - mean^2
                    sq = ln_pool.tile([128, 1024], BF, name=f"{_rep}_sq{lt}", tag="srcsb")
                    ssq = sc_pool.tile([128, 1], F32, name=f"{_rep}_ssq{lt}", tag="ssq")
                    nc.scalar.activation(sq[:], r[:], AF.Square, accum_out=ssq[:])
                    nc.vector.tensor_reduce(
                        rsum[:], r[:], axis=mybir.AxisListType.X, op=AL.add
                    )
                    mean = sc_pool.tile([128, 1], F32, name=f"{_rep}_mean{lt}", tag="mean")
                    nc.vector.tensor_scalar_mul(mean[:], rsum[:], 1.0 / DM)
                    m2 = sc_pool.tile([128, 1], F32, name=f"{_rep}_m2{lt}", tag="m2")
                    nc.vector.tensor_tensor(m2[:], mean[:], mean[:], AL.mult)
                    var = sc_pool.tile([128, 1], F32, name=f"{_rep}_var{lt}", tag="var")
                    nc.vector.tensor_scalar(
                        out=var[:], in0=ssq[:], scalar1=1.0 / DM, scalar2=m2[:],
                        op0=AL.mult, op1=AL.subtract,
                    )
                    std = sc_pool.tile([128, 1], F32, name=f"{_rep}_std{lt}", tag="std")
                    nc.scalar.activation(std[:], var[:], AF.Sqrt, bias=eps_sb[:])
                    rstd = sc_pool.tile([128, 1], F32, name=f"{_rep}_rstd{lt}", tag="rstd")
                    nc.vector.reciprocal(rstd[:], std[:])
                    # xn = (r - mean) * rstd in one fused op; g1/beta1 are folded
                    # into the transpose evictions (per-partition scale/bias)
                    xn = xres.tile([128, 1024], F32, name=f"x{_rep}_{lt}", tag=f"x{lt}")
                    x_tiles.append(xn)
                    nc.vector.tensor_scalar(
                        out=xn[:], in0=r[:], scalar1=mean[:], scalar2=rstd[:],
                        op0=AL.subtract, op1=AL.mult,
                    )
                    x_bf = ln_pool.tile([128, 1024], BF, name=f"{_rep}_xbf{lt}", tag="srcsb")
                    nc.vector.tensor_copy(x_bf[:], xn[:])
                    for c in range(NC_DM):
                        tp = tp_psp.tile([128, 128], BF, name=f"{_rep}_tp{lt}_{c}", tag="tp")
                        nc.tensor.transpose(tp[:], x_bf[:, c * 128 : (c + 1) * 128], ident[:])
                        nc.scalar.activation(
                            xT_sb[:, c * LC + lt * 128 : c * LC + (lt + 1) * 128],
                            tp[:],
                            AF.Identity,
                            bias=be1t_sb[:, c : c + 1],
                            scale=g1t_sb[:, c : c + 1],
                        )

            # ---------------- phase C: FFN + LN2 ----------------
            with ExitStack() as pc:
                ff_psp = pc.enter_context(tc.tile_pool(name="ff_ps", bufs=2, space="PSUM"))
                o_psp = pc.enter_context(tc.tile_pool(name="o_ps", bufs=1, space="PSUM"))
                ff1_sb = xtp.tile([128, NFF * LC], BF, name=f"ff1_{_rep}", tag="ff1")
                outps = [
                    o_psp.tile([128, 512], F32, name=f"{_rep}_ops{i}", tag=f"ops{i}")
                    for i in range(4)
                ]
                xT3 = xT_sb.rearrange("p (k l) -> p k l", l=LC)
                for cc in range(NFF):
                    if cc < len(w1_early):
                        w1t = w1_early[cc]
                    else:
                        w1t = w1_pool.tile(
                            [128, 1024], BF, name=f"{_rep}_w1t{cc}", tag="w1t"
                        )
                        nc.sync.dma_start(w1t[:], w1_in[cc])
                    ffps = ff_psp.tile([128, LC], F32, name=f"{_rep}_ffps{cc}", tag="ffps")
                    for k in range(NC_DM):
                        nc.tensor.matmul(
                            ffps[:],
                            w1t[:, k * 128 : (k + 1) * 128],
                            xT_sb[:, k * LC : (k + 1) * LC],
                            start=(k == 0),
                            stop=(k == NC_DM - 1),
                        )
                    nc.scalar.activation(
                        ff1_sb[:, cc * LC : (cc + 1) * LC],
                        ffps[:],
                        AF.Gelu,
                        bias=b1_sb[:, cc : cc + 1],
                    )

                ln2_pool = pc.enter_context(tc.tile_pool(name="ln2_pool", bufs=1))
                s2_pool = pc.enter_context(tc.tile_pool(name="s2_pool", bufs=2))
                # residual = g1*xn + (beta1 + b2), computed in the FFN matmul
                # shadow (x_tiles hold pre-affine xn)
                xb2s = []
                for lt in range(NLT):
                    xb2a = ln2_pool.tile(
                        [128, 1024], F32, name=f"{_rep}_xb2a_{lt}", tag=f"xb2a{lt}"
                    )
                    nc.vector.tensor_tensor(xb2a[:], x_tiles[lt][:], g1_rep[:], AL.mult)
                    xb2 = ln2_pool.tile(
                        [128, 1024], F32, name=f"{_rep}_xb2_{lt}", tag=f"xb2{lt}"
                    )
                    nc.vector.tensor_tensor(xb2[:], xb2a[:], bb2_rep[:], AL.add)
                    xb2s.append(xb2)
                for lt in range(NLT):
                    for cc in range(NFF):
                        for h in range(2):
                            nc.tensor.matmul(
                                outps[lt * 2 + h][:],
                                ff1_sb[:, cc * LC + lt * 128 : cc * LC + (lt + 1) * 128],
                                w2_sb[:, cc * 1024 + h * 512 : cc * 1024 + h * 512 + 512],
                                start=(cc == 0),
                                stop=(cc == NFF - 1),
                            )
                    r2 = ln2_pool.tile([128, 1024], F32, name=f"{_rep}_r2_{lt}", tag="r2")
                    for h in range(2):
                        nc.vector.tensor_tensor(
                            r2[:, h * 512 : (h + 1) * 512],
                            xb2s[lt][:, h * 512 : (h + 1) * 512],
                            outps[lt * 2 + h][:],
                            AL.add,
                        )
                    # mean/var in parallel on DVE/ACT: var = E[r^2] - mean^2
                    sq2 = ln2_pool.tile([128, 1024], F32, name=f"{_rep}_sq2_{lt}", tag="sq2")
                    ssq2 = s2_pool.tile([128, 1], F32, name=f"{_rep}_ssq2_{lt}", tag="ssq")
                    nc.scalar.activation(sq2[:], r2[:], AF.Square, accum_out=ssq2[:])
                    rsum2 = s2_pool.tile([128, 1], F32, name=f"{_rep}_rsum2_{lt}", tag="rsum")
                    nc.vector.tensor_reduce(
                        rsum2[:], r2[:], axis=mybir.AxisListType.X, op=AL.add
                    )
                    mean2 = s2_pool.tile([128, 1], F32, name=f"{_rep}_mean2_{lt}", tag="mean")
                    nc.vector.tensor_scalar_mul(mean2[:], rsum2[:], 1.0 / DM)
                    m22 = s2_pool.tile([128, 1], F32, name=f"{_rep}_m22_{lt}", tag="m2")
                    nc.vector.tensor_tensor(m22[:], mean2[:], mean2[:], AL.mult)
                    var2 = s2_pool.tile([128, 1], F32, name=f"{_rep}_var2_{lt}", tag="var")
                    nc.vector.tensor_scalar(
                        out=var2[:], in0=ssq2[:], scalar1=1.0 / DM, scalar2=m22[:],
                        op0=AL.mult, op1=AL.subtract,
                    )
                    std2 = s2_pool.tile([128, 1], F32, name=f"{_rep}_std2_{lt}", tag="std")
                    nc.scalar.activation(std2[:], var2[:], AF.Sqrt, bias=eps_sb[:])
                    rstd2 = s2_pool.tile([128, 1], F32, name=f"{_rep}_rstd2_{lt}", tag="rstd")
                    nc.vector.reciprocal(rstd2[:], std2[:])
                    xn2 = ln2_pool.tile([128, 1024], F32, name=f"{_rep}_xn2_{lt}", tag="r2x")
                    nc.vector.tensor_scalar(
                        out=xn2[:], in0=r2[:], scalar1=mean2[:], scalar2=rstd2[:],
                        op0=AL.subtract, op1=AL.mult,
                    )
                    t2 = ln2_pool.tile([128, 1024], F32, name=f"{_rep}_t2_{lt}", tag="sq2")
                    nc.vector.tensor_tensor(t2[:], xn2[:], g2_rep[:], AL.mult)
                    y = ln2_pool.tile([128, 1024], F32, name=f"{_rep}_y{lt}", tag="y")
                    nc.vector.tensor_tensor(y[:], t2[:], be2_rep[:], AL.add)
                    nc.sync.dma_start(out[lt * 128 : (lt + 1) * 128, :], y[:])

    nc.compile()
    return nc


def _prep_core(src_c, tgt_c, mask_c, W):
    """Host-side layout prep for one core's shard.  Returns the in_map dict."""
    bf = ml_dtypes.bfloat16
    X = np.ascontiguousarray(tgt_c.reshape(LC * D, DM))

    # xt: [NT, 128, NC_DM*128] fp8; [t, p, c*128+m] = X[t*128+m, c*128+p]
    xt = np.ascontiguousarray(
        X.reshape(NT, 128, NC_DM, 128).transpose(0, 3, 2, 1).reshape(NT, 128, NC_DM * 128)
    ).astype(F8NP)

    def wprep_f8(Wm):
        # [128, NC_DM*1024] fp8; [p, c*1024+n] = (W*FP8_SCALE)[c*128+p, n]
        Wp = (Wm * FP8_SCALE).astype(np.float32)
        return np.ascontiguousarray(
            Wp.reshape(NC_DM, 128, DM).transpose(1, 0, 2).reshape(128, NC_DM * 1024)
        ).astype(F8NP)

    # st: [128, NC_DM*LC] fp8; [p, c*LC+f] = src_c[f, c*128+p]
    st = np.ascontiguousarray(
        src_c.reshape(LC, NC_DM, 128).transpose(2, 1, 0).reshape(128, NC_DM * LC)
    )
    # wkt: [64, (h*NC_DM+c)*128+n] = 8*Wk[c*128+n, h*64+d]
    wkt = np.ascontiguousarray(
        (W["Wk"] * FP8_SCALE)
        .reshape(NC_DM, 128, H, DH)
        .transpose(3, 2, 0, 1)
        .reshape(64, H * NC_DM * 128)
    ).astype(F8NP)
    bqp = np.ascontiguousarray(
        W["bq"].reshape(H, DH).T * QT_SCALE
    ).astype(np.float32)

    w1p = np.ascontiguousarray(
        W["W1"].reshape(NC_DM, 128, NFF, 128).transpose(2, 1, 0, 3).reshape(NFF, 128, 1024)
    ).astype(bf)
    # w2p: [128, NFF*1024]; [p, cc*1024+n] = W2[cc*128+p, n] (one resident DMA)
    w2p = np.ascontiguousarray(
        W["W2"].reshape(NFF, 128, DM).transpose(1, 0, 2).reshape(128, NFF * 1024)
    ).astype(bf)

    # wq h-major: [p, h*512 + c*64 + d] = 8*Wq[c*128+p, h*64+d] so phase Q can
    # start after a single 64KB h-slice DMA instead of the full 1MB tensor
    wqh = np.ascontiguousarray(
        (W["Wq"] * FP8_SCALE)
        .reshape(NC_DM, 128, H, 64)
        .transpose(1, 2, 0, 3)
        .reshape(128, NC_DM * 1024)
    ).astype(F8NP)
    return {
        "xt": xt,
        "wv": wprep_f8(W["Wv"]),
        "wq": wqh,
        "st": st.astype(F8NP),
        "wkt": wkt,
        "bqp": bqp,
        # bv folds into the residual: src + bv (probs sum to 1)
        "srcr": np.ascontiguousarray(src_c + W["bv"][None, :]).astype(np.float32),
        "maskp": np.ascontiguousarray(mask_c.reshape(NT, 128).T).astype(np.float32),
        "b1p": np.ascontiguousarray(W["b1"].reshape(NFF, 128).T).astype(np.float32),
        "bb2p": (W["beta1"] + W["b2"]).reshape(1, DM).astype(bf),
        "g1p": W["g1"].reshape(1, DM).astype(bf),
        "g1tp": np.ascontiguousarray(W["g1"].reshape(NC_DM, 128).T).astype(np.float32),
        "be1tp": np.ascontiguousarray(W["beta1"].reshape(NC_DM, 128).T).astype(np.float32),
        "g2p": W["g2"].reshape(1, DM).astype(bf),
        "be2p": W["beta2"].reshape(1, DM).astype(bf),
        "w1p": w1p,
        "w2p": w2p,
    }


def make_in_maps(**inputs):
    inp = {k: np.asarray(v) for k, v in inputs.items()}
    W = {
        k: inp[k]
        for k in ("Wq", "bq", "Wk", "bk", "Wv", "bv", "W1", "b1", "W2", "b2",
                  "g1", "beta1", "g2", "beta2")
    }
    in_maps = []
    for c in range(NCORES):
        sl = slice(c * LC, (c + 1) * LC)
        in_maps.append(_prep_core(inp["src"][sl], inp["target"][sl], inp["attn_mask"][sl], W))
    return in_maps


def get_nc(repeat=1):
    key = ("nc", repeat)
    if key not in _CACHE:
        _CACHE[key] = _build_nc(repeat)
    return _CACHE[key]


def kernel(**inputs) -> np.ndarray:
    nc = get_nc()
    in_maps = make_in_maps(**inputs)
    res = run_bass_kernel_spmd(nc, in_maps, core_ids=list(range(NCORES)))
    return np.concatenate([res.results[c]["out"] for c in range(NCORES)], axis=0)


if __name__ == "__main__":
    import reference

    inputs = {k: np.asarray(v) for k, v in reference.setup_inputs().items()}
    got = kernel(**inputs)
    exp = np.asarray(reference.reference(**inputs))
    err = np.abs(got - exp).max() / np.abs(exp).max()
    print("Relative error:", err)



# revision 34
# speedup vs baseline: 1.0142x; 1.0142x over previous
"""Trainium2 Bass kernel for nn_ContextEncoderLayer (per-position cross-attention
encoder layer).  Shards the sequence dim L across 8 NeuronCores; each core runs an
identical Bass/Tile program on its 256-position shard.

Layout strategy (per core, L_C=256 positions):
  - attention works on (l,d)-row tiles: 64 tiles of [128 partitions = 4 positions x 32
    candidates, free = (h,dh) = 1024].
  - LOW-RANK K PATH: instead of projecting K for all 8192 rows, compute
    qT = (src@Wq+bq)^T (phase Q), then qWT = qT @ Wk^T per head (phase W, fp8,
    32x scale) and get scores directly as one fused PE pass per tile:
    scores[ld, (h,l')] = x_tile^T-chunk-pairs (stationary, shared with the V
    projection) x qWT pair-slices (fp8 DoubleRow).  Off-diagonal (h,l') slots
    are masked with a -30000 constant before exp; attn_mask rides the ACT Exp
    per-partition bias; a 4-slot DVE reduce yields ex[ld, h].
  - V projection: fp8 DoubleRow PE matmuls, x^T chunk-pairs stationary, fp8 Wv
    (8x host prescale) streamed; bv folded into the residual (src + bv).
  - candidate sum + softmax denominator: fp8 DoubleRow selector matmuls over
    tile PAIRS (interleaved pv/ex streams, shifted 0/1 and 0/8 selector banks).
  - LN1/LN2: var = E[x^2]-mean^2 with DVE reduce || ACT Square-accum in
    parallel, fused (r-mean)*rstd tensor_scalar; g1/beta1 applied inside the
    x^T transpose evictions (per-partition ACT scale/bias APs).
  - FFN: W1/W2 bf16 (fp8 FFN fails the 2e-2 gate), W2 resident + W1 prefetched
    during phase B, GELU+bias fused into the PSUM->SBUF eviction; output fp32.
"""

import sys

sys.path.insert(0, "/opt/trn_rl_repo")

from contextlib import ExitStack

import numpy as np
import ml_dtypes

import concourse.bacc as bacc
import concourse.tile as tile
from concourse import mybir
from concourse.bass_utils import run_bass_kernel_spmd
from concourse.masks import make_identity

L, D, DM, H, FF = 2048, 32, 1024, 16, 4096
DH = DM // H  # 64
SCALE = float(np.sqrt(DH))  # 8.0
NCORES = 8
LC = L // NCORES  # 256 positions per core
NT = LC * D // 128  # 64 (l,d)-row tiles per core
NLT = LC // 128  # 2 l-tiles per core
NC_DM = DM // 128  # 8 dm chunks
CH = NC_DM + 1  # 9: 8 chunks + ones-row (bias fold, Q path only)
NFF = FF // 128  # 32 ff chunks
TPB = NT // NLT  # 32 (l,d)-tiles per l-tile
BF = mybir.dt.bfloat16
F32 = mybir.dt.float32
F8 = mybir.dt.float8e4
F8NP = ml_dtypes.float8_e4m3
FP8_SCALE = 8.0  # host pre-scale on Wk/Wv so fp8 weights use the normal range
QT_SCALE = 4.0  # q carried at 4x (bf16) into the qW GEMM
QW_NET = FP8_SCALE * QT_SCALE  # qWT is stored at 32x true qW

_CACHE = {}
PENDING_DELAY = True


def _sel_matrices(value=1.0):
    """Shifted selector-pair bank [128, 2, 256] (256 stride: dual-fp8 LDW
    requires power-of-2 chunk strides).

    Slice [:, :, 120-8*jp : 248-8*jp] is the DoubleRow lhsT for tile pair
    (2jp, 2jp+1) of an l-tile: slice[p, two, c] = value iff
    c == 8*jp + 4*two + p//32  (ctx partition 4*tt + p//32 for tile tt)."""
    g = np.zeros((128, 2, 256), dtype=np.float32)
    for p in range(128):
        for two in range(2):
            g[p, two, 120 + 4 * two + p // 32] = value
    return g.reshape(128, 2 * 256).astype(F8NP)


def _negdiag():
    """[128, 64] additive pre-exp mask: 0 on a partition's own l-slot
    (h*4 + p//32), -30000 on the other tiles' slots (garbage scores)."""
    m = np.full((128, 64), -30000.0, dtype=np.float32)
    for p in range(128):
        for h in range(16):
            m[p, h * 4 + p // 32] = 0.0
    return m


def _build_nc(repeat=1):
    nc = bacc.Bacc("TRN2", target_bir_lowering=False, debug=False, num_devices=NCORES)

    # ---------------- I/O ----------------
    xt_in = nc.dram_tensor("xt", [NT, 128, NC_DM * 128], F8, kind="ExternalInput")
    wv_in = nc.dram_tensor("wv", [128, NC_DM * 1024], F8, kind="ExternalInput")
    wq_in = nc.dram_tensor("wq", [128, NC_DM * 1024], F8, kind="ExternalInput")
    st_in = nc.dram_tensor("st", [128, NC_DM * LC], F8, kind="ExternalInput")
    wkt_in = nc.dram_tensor("wkt", [64, H * NC_DM * 128], F8, kind="ExternalInput")
    bq_in = nc.dram_tensor("bqp", [64, H], F32, kind="ExternalInput")
    src_in = nc.dram_tensor("srcr", [LC, DM], F32, kind="ExternalInput")
    mask_in = nc.dram_tensor("maskp", [128, NT], F32, kind="ExternalInput")
    b1_in = nc.dram_tensor("b1p", [128, NFF], F32, kind="ExternalInput")
    bb2_in = nc.dram_tensor("bb2p", [1, DM], BF, kind="ExternalInput")
    g1_in = nc.dram_tensor("g1p", [1, DM], BF, kind="ExternalInput")
    g1t_in = nc.dram_tensor("g1tp", [128, NC_DM], F32, kind="ExternalInput")
    be1t_in = nc.dram_tensor("be1tp", [128, NC_DM], F32, kind="ExternalInput")
    g2_in = nc.dram_tensor("g2p", [1, DM], BF, kind="ExternalInput")
    be2_in = nc.dram_tensor("be2p", [1, DM], BF, kind="ExternalInput")
    w1_in = nc.dram_tensor("w1p", [NFF, 128, 1024], BF, kind="ExternalInput")
    w2_in = nc.dram_tensor("w2p", [128, NFF * 1024], BF, kind="ExternalInput")
    out = nc.dram_tensor("out", [LC, DM], F32, kind="ExternalOutput")

    g_const = nc.inline_tensor(np.asarray(_sel_matrices(1.0)), name="gsel")
    g8_const = nc.inline_tensor(np.asarray(_sel_matrices(FP8_SCALE)), name="g8sel")
    negd_const = nc.inline_tensor(np.asarray(_negdiag()), name="negd")

    AL = mybir.AluOpType
    AF = mybir.ActivationFunctionType
    DR = mybir.MatmulPerfMode.DoubleRow

    with tile.TileContext(nc) as tc, ExitStack() as top:
        consts = top.enter_context(tc.tile_pool(name="consts", bufs=1))

        # ------- constants / params resident in SBUF (Q-phase deps first) -------
        wq_sb = consts.tile([128, NC_DM * 1024], F8)
        nc.sync.dma_start(wq_sb[:], wq_in[:])
        st_sb = consts.tile([128, NC_DM * LC], F8)
        nc.sync.dma_start(st_sb[:], st_in[:])
        wkt_sb = consts.tile([64, H * NC_DM * 128], F8)
        nc.sync.dma_start(wkt_sb[:], wkt_in[:])
        bq_sb = consts.tile([64, H], F32)
        nc.scalar.dma_start(bq_sb[:], bq_in[:])
        negd_sb = consts.tile([128, 64], F32)
        nc.scalar.dma_start(negd_sb[:], negd_const[:])
        wv_sb = consts.tile([128, NC_DM * 1024], F8)
        nc.sync.dma_start(wv_sb[:], wv_in[:])
        qWT_sb = consts.tile([128, NC_DM * NT * 64], F8)
        mask_sb = consts.tile([128, NT], F32)
        nc.scalar.dma_start(mask_sb[:], mask_in[:])
        b1_sb = consts.tile([128, NFF], F32)
        nc.scalar.dma_start(b1_sb[:], b1_in[:])
        g_sb = consts.tile([128, 2 * 256], F8)
        nc.scalar.dma_start(g_sb[:], g_const[:])
        g8_sb = consts.tile([128, 2 * 256], F8)
        nc.scalar.dma_start(g8_sb[:], g8_const[:])
        # big resident W2 tile; its DMA is issued mid-phase-B (ACT ring) so the
        # transfer neither blocks the SP ring nor eats startup DMA bandwidth
        w2_sb = consts.tile([128, NFF * 1024], BF)
        ident = consts.tile([128, 128], BF)
        make_identity(nc, ident[:])
        eps_sb = consts.tile([128, 1], F32)
        nc.vector.memset(eps_sb[:], 1e-5)

        def rep128(name, src):  # [1, DM] -> [128, DM] partition-broadcast, bf16
            t = consts.tile([128, DM], BF, name=name)
            nc.gpsimd.dma_start(t[:], src[0:1, :].broadcast_to([128, DM]))
            return t

        g1_rep = rep128("g1_rep", g1_in)
        bb2_rep = rep128("bb2_rep", bb2_in)
        g2_rep = rep128("g2_rep", g2_in)
        be2_rep = rep128("be2_rep", be2_in)
        g1t_sb = consts.tile([128, NC_DM], F32)
        nc.scalar.dma_start(g1t_sb[:], g1t_in[:])
        be1t_sb = consts.tile([128, NC_DM], F32)
        nc.scalar.dma_start(be1t_sb[:], be1t_in[:])

        xres = top.enter_context(tc.tile_pool(name="xres", bufs=1))
        xtp = top.enter_context(tc.tile_pool(name="xtp", bufs=1))
        w1_pool = top.enter_context(tc.tile_pool(name="w1_pool", bufs=3))

        for _rep in range(repeat):
            x_tiles = []
            w1_early = []
            xT_sb = xtp.tile([128, NC_DM * LC], BF, name=f"xT{_rep}", tag="xT")
            # ------- phase Q: qT[dh, (h,l)] = QT_SCALE*(src @ Wq + bq)^T, bf16 -------
            qtpool = ExitStack()
            qtp = qtpool.enter_context(tc.tile_pool(name="qtp", bufs=1))
            qT_sb = qtp.tile([64, H * LC], BF, name=f"qT{_rep}", tag="qT")
            with tc.tile_pool(name="qps", bufs=2, space="PSUM") as qpsp:
                st3 = st_sb.rearrange("p (c f) -> p c f", f=LC)
                wq3 = wq_sb.rearrange("p (c n) -> p c n", n=1024)
                for h in range(H):
                    qps = qpsp.tile([64, LC], F32, name=f"qps{_rep}_{h}", tag="qps")
                    for pc in range(NC_DM // 2):
                        nc.tensor.matmul(
                            qps[:],
                            wq3[:, 2 * pc : 2 * pc + 2, h * 64 : (h + 1) * 64],
                            st3[:, 2 * pc : 2 * pc + 2, :],
                            start=(pc == 0),
                            stop=(pc == NC_DM // 2 - 1),
                            perf_mode=DR,
                        )
                    nc.scalar.activation(
                        qT_sb[:, h * LC : (h + 1) * LC],
                        qps[:],
                        AF.Identity,
                        bias=bq_sb[:, h : h + 1],
                        scale=QT_SCALE / FP8_SCALE,
                    )
            # ------- phase W: qWT[n, (h, tile, l')] = 32x qW = q @ Wk^T, fp8 -------
            with tc.tile_pool(name="qwps", bufs=3, space="PSUM") as qwpsp:
                qw5 = qWT_sb.rearrange("p (c h t f) -> p c h t f", h=H, t=NT, f=4)
                for h in range(H):
                    for c4 in range(NC_DM // 4):
                        qwps = qwpsp.tile(
                            [128, 4 * LC], F32, name=f"qw{_rep}_{h}_{c4}", tag="qwps"
                        )
                        for cj in range(4):
                            c = 4 * c4 + cj
                            nc.tensor.matmul(
                                qwps[:, cj * LC : (cj + 1) * LC],
                                wkt_sb[
                                    :, (h * NC_DM + c) * 128 : (h * NC_DM + c + 1) * 128
                                ],
                                qT_sb[:, h * LC : (h + 1) * LC],
                                start=True,
                                stop=True,
                            )
                        # one wide strided eviction per 4 chunks, alternating
                        # engines so the PE is not eviction-throttled
                        dst = qw5[:, 4 * c4 : 4 * c4 + 4, h, :, :]
                        if (h * 2 + c4) % 2 == 0:
                            nc.scalar.activation(dst, qwps[:], AF.Copy)
                        else:
                            nc.vector.tensor_copy(dst, qwps[:])
            qtpool.close()

            # ---------------- phase B: projections + attention ----------------
            with ExitStack() as pb:
                proj_ps = pb.enter_context(tc.tile_pool(name="proj_ps", bufs=3, space="PSUM"))
                acc_ps = pb.enter_context(tc.tile_pool(name="acc_ps", bufs=1, space="PSUM"))
                den_psp = pb.enter_context(tc.tile_pool(name="den_ps", bufs=1, space="PSUM"))
                tp_psp = pb.enter_context(tc.tile_pool(name="tp_ps", bufs=1, space="PSUM"))
                sc_psp = pb.enter_context(tc.tile_pool(name="sc_ps", bufs=1, space="PSUM"))
                xt_pool = pb.enter_context(tc.tile_pool(name="xt_pool", bufs=5))
                kb_pool = pb.enter_context(tc.tile_pool(name="kb_pool", bufs=3))
                t_pool = pb.enter_context(tc.tile_pool(name="t_pool", bufs=2))
                pv_pool = pb.enter_context(tc.tile_pool(name="pv_pool", bufs=4))
                sc_pool = pb.enter_context(tc.tile_pool(name="sc_pool", bufs=3))
                ln_pool = pb.enter_context(tc.tile_pool(name="ln_pool", bufs=1))

                def proj_half(ps, w_sb, xt3, h):
                    # fp8 DoubleRow: 4 chunk-pairs for one 512-wide output half
                    w3 = w_sb.rearrange("p (c n) -> p c n", n=1024)
                    for pc in range(NC_DM // 2):
                        nc.tensor.matmul(
                            ps[:],
                            xt3[:, 2 * pc : 2 * pc + 2, :],
                            w3[:, 2 * pc : 2 * pc + 2, h * 512 : (h + 1) * 512],
                            start=(pc == 0),
                            stop=(pc == NC_DM // 2 - 1),
                            perf_mode=DR,
                        )

                pending = []

                g3 = g_sb.rearrange("p (two c) -> p two c", c=256)
                g83 = g8_sb.rearrange("p (two c) -> p two c", c=256)

                def emit_sel(p):
                    # one fp8 DoubleRow matmul per tile PAIR (2jp, 2jp+1): the
                    # two tiles' pv/ex stream interleaved; shifted selector-pair
                    # slice as lhsT maps rows to ctx partitions 4*tt + p//32
                    (pvp, exp2, jp, ctx_ps, den_ps) = p
                    off = 120 - 8 * jp
                    pv3 = pvp.rearrange("p (two f) -> p two f", two=2)
                    for h in range(2):
                        nc.tensor.matmul(
                            ctx_ps[:, h * 512 : (h + 1) * 512],
                            g3[:, :, off : off + 128],
                            pv3[:, :, h * 512 : (h + 1) * 512],
                            start=(jp == 0),
                            stop=(jp == TPB // 2 - 1),
                            perf_mode=DR,
                        )
                    nc.tensor.matmul(
                        den_ps[:],
                        g83[:, :, off : off + 128],
                        exp2.rearrange("p (two f) -> p two f", two=2),
                        start=(jp == 0),
                        stop=(jp == TPB // 2 - 1),
                        perf_mode=DR,
                    )

                qw6 = qWT_sb.rearrange("p (c h t f) -> p c h t f", h=H, t=NT, f=4)
                for lt in range(NLT):
                    ctx_ps = acc_ps.tile([128, 1024], F32, name=f"{_rep}_ctx_ps{lt}", tag="ctx")
                    den_ps = den_psp.tile([128, 16], F32, name=f"{_rep}_den_ps{lt}", tag="den")
                    for tt in range(TPB):
                        t = lt * TPB + tt
                        if lt == 0 and tt >= 3 and tt % 4 == 3:
                            s = tt // 4  # 0..7: stream W2 in 1MB slices between xt loads
                            nc.sync.dma_start(
                                w2_sb[:, s * 4096 : (s + 1) * 4096],
                                w2_in[:, s * 4096 : (s + 1) * 4096],
                            )
                        if lt == 1 and tt >= 26 and tt % 2 == 0:
                            cc_pre = (tt - 26) // 2  # 0..2: prefetch first W1 chunks
                            w1e = w1_pool.tile(
                                [128, 1024], BF, name=f"{_rep}_w1t{cc_pre}", tag="w1t"
                            )
                            nc.sync.dma_start(w1e[:], w1_in[cc_pre])
                            w1_early.append(w1e)
                        xt_sb = xt_pool.tile([128, NC_DM * 128], F8, name=f"{_rep}_xt{t}", tag="xt")
                        nc.sync.dma_start(xt_sb[:], xt_in[t])
                        xt3 = xt_sb.rearrange("p (c m) -> p c m", m=128)
                        # scores[ld, (h,l')] on PE: xt chunk-pairs stationary,
                        # qWT pair-slices streamed (32x true scale)
                        scps = sc_psp.tile([128, 64], F32, name=f"{_rep}_scp{t}", tag="scps")
                        for pc in range(NC_DM // 2):
                            nc.tensor.matmul(
                                scps[:],
                                xt3[:, 2 * pc : 2 * pc + 2, :],
                                qw6[:, 2 * pc : 2 * pc + 2, :, t, :],
                                start=(pc == 0),
                                stop=(pc == NC_DM // 2 - 1),
                                perf_mode=DR,
                            )
                        if PENDING_DELAY and len(pending) >= 2:
                            emit_sel(pending.pop(0))
                        vb = kb_pool.tile([128, 1024], F8, name=f"{_rep}_vb{t}", tag="vb")
                        for hh in range(2):
                            vpsh = proj_ps.tile(
                                [128, 512], F32, name=f"{_rep}_vps{t}_{hh}", tag="proj"
                            )
                            proj_half(vpsh, wv_sb, xt3, hh)
                            if hh == 0:
                                nc.vector.tensor_copy(vb[:, 0:512], vpsh[:])
                            else:
                                nc.scalar.copy(vb[:, 512:1024], vpsh[:])
                        scm = t_pool.tile([128, 64], F32, name=f"{_rep}_scm{t}", tag="t")
                        nc.vector.tensor_tensor(scm[:], scps[:], negd_sb[:], AL.add)
                        ex64 = t_pool.tile([128, 64], BF, name=f"{_rep}_e64{t}", tag="e64")
                        nc.scalar.activation(
                            ex64[:],
                            scm[:],
                            AF.Exp,
                            bias=mask_sb[:, t : t + 1],
                            scale=1.0 / (QW_NET * SCALE),
                        )
                        if tt % 2 == 0:
                            pvp = pv_pool.tile([128, 2048], F8, name=f"{_rep}_pv{t}", tag="pv")
                            exp2 = sc_pool.tile([128, 32], F8, name=f"{_rep}_ex{t}", tag="ex")
                        half = tt % 2
                        with nc.allow_low_precision(reason="4-slot sum, 3 are ~0"):
                            nc.vector.tensor_reduce(
                                exp2[:, half * 16 : half * 16 + 16],
                                ex64.rearrange("p (h x) -> p h x", x=4),
                                axis=mybir.AxisListType.X,
                                op=AL.add,
                            )
                        nc.vector.tensor_tensor(
                            pvp[:, half * 1024 : half * 1024 + 1024].rearrange(
                                "p (h x) -> p h x", x=DH
                            ),
                            vb.rearrange("p (h x) -> p h x", x=DH),
                            exp2[:, half * 16 : half * 16 + 16]
                            .rearrange("p (h o) -> p h o", o=1)
                            .broadcast_to([128, H, DH]),
                            AL.mult,
                        )
                        if tt % 2 == 1:
                            pending.append((pvp, exp2, tt // 2, ctx_ps, den_ps))
                    while pending:
                        emit_sel(pending.pop(0))

                    # ---- l-tile epilogue: softmax-normalize, residual, LN1, x^T ----
                    rd = sc_pool.tile([128, 16], F32, name=f"{_rep}_rd{lt}", tag="rd")
                    nc.vector.reciprocal(rd[:], den_ps[:])
                    ctxn = t_pool.tile([128, 1024], F32, name=f"{_rep}_ctxn{lt}", tag="t")
                    nc.vector.tensor_tensor(
                        ctxn.rearrange("p (h x) -> p h x", x=DH),
                        ctx_ps.rearrange("p (h x) -> p h x", x=DH),
                        rd.rearrange("p (h o) -> p h o", o=1).broadcast_to([128, H, DH]),
                        AL.mult,
                    )
                    src_sb = ln_pool.tile([128, 1024], F32, name=f"{_rep}_srcsb{lt}", tag="srcsb")
                    nc.sync.dma_start(src_sb[:], src_in[lt * 128 : (lt + 1) * 128, :])
                    r = ln_pool.tile([128, 1024], F32, name=f"{_rep}_r{lt}", tag="r")
                    rsum = sc_pool.tile([128, 1], F32, name=f"{_rep}_rsum{lt}", tag="rsum")
                    nc.vector.tensor_tensor(r[:], ctxn[:], src_sb[:], AL.add)
                    # mean/var in parallel on DVE/ACT: var = E[r^2] - mean^2
                    sq = ln_pool.tile([128, 1024], BF, name=f"{_rep}_sq{lt}", tag="srcsb")
                    ssq = sc_pool.tile([128, 1], F32, name=f"{_rep}_ssq{lt}", tag="ssq")
                    nc.scalar.activation(sq[:], r[:], AF.Square, accum_out=ssq[:])
                    nc.vector.tensor_reduce(
                        rsum[:], r[:], axis=mybir.AxisListType.X, op=AL.add
                    )
                    mean = sc_pool.tile([128, 1], F32, name=f"{_rep}_mean{lt}", tag="mean")
                    nc.vector.tensor_scalar_mul(mean[:], rsum[:], 1.0 / DM)
                    m2 = sc_pool.tile([128, 1], F32, name=f"{_rep}_m2{lt}", tag="m2")
                    nc.vector.tensor_tensor(m2[:], mean[:], mean[:], AL.mult)
                    var = sc_pool.tile([128, 1], F32, name=f"{_rep}_var{lt}", tag="var")
                    nc.vector.tensor_scalar(
                        out=var[:], in0=ssq[:], scalar1=1.0 / DM, scalar2=m2[:],
                        op0=AL.mult, op1=AL.subtract,
                    )
                    std = sc_pool.tile([128, 1], F32, name=f"{_rep}_std{lt}", tag="std")
                    nc.scalar.activation(std[:], var[:], AF.Sqrt, bias=eps_sb[:])
                    rstd = sc_pool.tile([128, 1], F32, name=f"{_rep}_rstd{lt}", tag="rstd")
                    nc.vector.reciprocal(rstd[:], std[:])
                    # xn = (r - mean) * rstd in one fused op; g1/beta1 are folded
                    # into the transpose evictions (per-partition scale/bias)
                    xn = xres.tile([128, 1024], F32, name=f"x{_rep}_{lt}", tag=f"x{lt}")
                    x_tiles.append(xn)
                    nc.vector.tensor_scalar(
                        out=xn[:], in0=r[:], scalar1=mean[:], scalar2=rstd[:],
                        op0=AL.subtract, op1=AL.mult,
                    )
                    x_bf = ln_pool.tile([128, 1024], BF, name=f"{_rep}_xbf{lt}", tag="srcsb")
                    nc.vector.tensor_copy(x_bf[:], xn[:])
                    for c in range(NC_DM):
                        tp = tp_psp.tile([128, 128], BF, name=f"{_rep}_tp{lt}_{c}", tag="tp")
                        nc.tensor.transpose(tp[:], x_bf[:, c * 128 : (c + 1) * 128], ident[:])
                        nc.scalar.activation(
                            xT_sb[:, c * LC + lt * 128 : c * LC + (lt + 1) * 128],
                            tp[:],
                            AF.Identity,
                            bias=be1t_sb[:, c : c + 1],
                            scale=g1t_sb[:, c : c + 1],
                        )

            # ---------------- phase C: FFN + LN2 ----------------
            with ExitStack() as pc:
                ff_psp = pc.enter_context(tc.tile_pool(name="ff_ps", bufs=2, space="PSUM"))
                o_psp = pc.enter_context(tc.tile_pool(name="o_ps", bufs=1, space="PSUM"))
                ff1_sb = xtp.tile([128, NFF * LC], BF, name=f"ff1_{_rep}", tag="ff1")
                outps = [
                    o_psp.tile([128, 512], F32, name=f"{_rep}_ops{i}", tag=f"ops{i}")
                    for i in range(4)
                ]
                xT3 = xT_sb.rearrange("p (k l) -> p k l", l=LC)
                for cc in range(NFF):
                    if cc < len(w1_early):
                        w1t = w1_early[cc]
                    else:
                        w1t = w1_pool.tile(
                            [128, 1024], BF, name=f"{_rep}_w1t{cc}", tag="w1t"
                        )
                        nc.sync.dma_start(w1t[:], w1_in[cc])
                    ffps = ff_psp.tile([128, LC], F32, name=f"{_rep}_ffps{cc}", tag="ffps")
                    for k in range(NC_DM):
                        nc.tensor.matmul(
                            ffps[:],
                            w1t[:, k * 128 : (k + 1) * 128],
                            xT_sb[:, k * LC : (k + 1) * LC],
                            start=(k == 0),
                            stop=(k == NC_DM - 1),
                        )
                    nc.scalar.activation(
                        ff1_sb[:, cc * LC : (cc + 1) * LC],
                        ffps[:],
                        AF.Gelu,
                        bias=b1_sb[:, cc : cc + 1],
                    )

                ln2_pool = pc.enter_context(tc.tile_pool(name="ln2_pool", bufs=1))
                s2_pool = pc.enter_context(tc.tile_pool(name="s2_pool", bufs=2))
                # residual = g1*xn + (beta1 + b2), computed in the FFN matmul
                # shadow (x_tiles hold pre-affine xn)
                xb2s = []
                for lt in range(NLT):
                    xb2a = ln2_pool.tile(
                        [128, 1024], F32, name=f"{_rep}_xb2a_{lt}", tag=f"xb2a{lt}"
                    )
                    nc.vector.tensor_tensor(xb2a[:], x_tiles[lt][:], g1_rep[:], AL.mult)
                    xb2 = ln2_pool.tile(
                        [128, 1024], F32, name=f"{_rep}_xb2_{lt}", tag=f"xb2{lt}"
                    )
                    nc.vector.tensor_tensor(xb2[:], xb2a[:], bb2_rep[:], AL.add)
                    xb2s.append(xb2)
                for lt in range(NLT):
                    for cc in range(NFF):
                        for h in range(2):
                            nc.tensor.matmul(
                                outps[lt * 2 + h][:],
                                ff1_sb[:, cc * LC + lt * 128 : cc * LC + (lt + 1) * 128],
                                w2_sb[:, cc * 1024 + h * 512 : cc * 1024 + h * 512 + 512],
                                start=(cc == 0),
                                stop=(cc == NFF - 1),
                            )
                    r2 = ln2_pool.tile([128, 1024], F32, name=f"{_rep}_r2_{lt}", tag="r2")
                    for h in range(2):
                        nc.vector.tensor_tensor(
                            r2[:, h * 512 : (h + 1) * 512],
                            xb2s[lt][:, h * 512 : (h + 1) * 512],
                            outps[lt * 2 + h][:],
                            AL.add,
                        )
                    # mean/var in parallel on DVE/ACT: var = E[r^2] - mean^2
                    sq2 = ln2_pool.tile([128, 1024], F32, name=f"{_rep}_sq2_{lt}", tag="sq2")
                    ssq2 = s2_pool.tile([128, 1], F32, name=f"{_rep}_ssq2_{lt}", tag="ssq")
                    nc.scalar.activation(sq2[:], r2[:], AF.Square, accum_out=ssq2[:])
                    rsum2 = s2_pool.tile([128, 1], F32, name=f"{_rep}_rsum2_{lt}", tag="rsum")
                    nc.vector.tensor_reduce(
                        rsum2[:], r2[:], axis=mybir.AxisListType.X, op=AL.add
                    )
                    mean2 = s2_pool.tile([128, 1], F32, name=f"{_rep}_mean2_{lt}", tag="mean")
                    nc.vector.tensor_scalar_mul(mean2[:], rsum2[:], 1.0 / DM)
                    m22 = s2_pool.tile([128, 1], F32, name=f"{_rep}_m22_{lt}", tag="m2")
                    nc.vector.tensor_tensor(m22[:], mean2[:], mean2[:], AL.mult)
                    var2 = s2_pool.tile([128, 1], F32, name=f"{_rep}_var2_{lt}", tag="var")
                    nc.vector.tensor_scalar(
                        out=var2[:], in0=ssq2[:], scalar1=1.0 / DM, scalar2=m22[:],
                        op0=AL.mult, op1=AL.subtract,
                    )
                    std2 = s2_pool.tile([128, 1], F32, name=f"{_rep}_std2_{lt}", tag="std")
                    nc.scalar.activation(std2[:], var2[:], AF.Sqrt, bias=eps_sb[:])
                    rstd2 = s2_pool.tile([128, 1], F32, name=f"{_rep}_rstd2_{lt}", tag="rstd")
                    nc.vector.reciprocal(rstd2[:], std2[:])
                    xn2 = ln2_pool.tile([128, 1024], F32, name=f"{_rep}_xn2_{lt}", tag="r2x")
                    nc.vector.tensor_scalar(
                        out=xn2[:], in0=r2[:], scalar1=mean2[:], scalar2=rstd2[:],
                        op0=AL.subtract, op1=AL.mult,
                    )
                    t2 = ln2_pool.tile([128, 1024], F32, name=f"{_rep}_t2_{lt}", tag="sq2")
                    nc.vector.tensor_tensor(t2[:], xn2[:], g2_rep[:], AL.mult)
                    y = ln2_pool.tile([128, 1024], F32, name=f"{_rep}_y{lt}", tag="y")
                    nc.vector.tensor_tensor(y[:], t2[:], be2_rep[:], AL.add)
                    nc.sync.dma_start(out[lt * 128 : (lt + 1) * 128, :], y[:])

    nc.compile()
    return nc


def _prep_core(src_c, tgt_c, mask_c, W):
    """Host-side layout prep for one core's shard.  Returns the in_map dict."""
    bf = ml_dtypes.bfloat16
    X = np.ascontiguousarray(tgt_c.reshape(LC * D, DM))

    # xt: [NT, 128, NC_DM*128] fp8; [t, p, c*128+m] = X[t*128+m, c*128+p]
    xt = np.ascontiguousarray(
        X.reshape(NT, 128, NC_DM, 128).transpose(0, 3, 2, 1).reshape(NT, 128, NC_DM * 128)
    ).astype(F8NP)

    def wprep_f8(Wm):
        # [128, NC_DM*1024] fp8; [p, c*1024+n] = (W*FP8_SCALE)[c*128+p, n]
        Wp = (Wm * FP8_SCALE).astype(np.float32)
        return np.ascontiguousarray(
            Wp.reshape(NC_DM, 128, DM).transpose(1, 0, 2).reshape(128, NC_DM * 1024)
        ).astype(F8NP)

    # st: [128, NC_DM*LC] fp8; [p, c*LC+f] = src_c[f, c*128+p]
    st = np.ascontiguousarray(
        src_c.reshape(LC, NC_DM, 128).transpose(2, 1, 0).reshape(128, NC_DM * LC)
    )
    # wkt: [64, (h*NC_DM+c)*128+n] = 8*Wk[c*128+n, h*64+d]
    wkt = np.ascontiguousarray(
        (W["Wk"] * FP8_SCALE)
        .reshape(NC_DM, 128, H, DH)
        .transpose(3, 2, 0, 1)
        .reshape(64, H * NC_DM * 128)
    ).astype(F8NP)
    bqp = np.ascontiguousarray(
        W["bq"].reshape(H, DH).T * QT_SCALE
    ).astype(np.float32)

    w1p = np.ascontiguousarray(
        W["W1"].reshape(NC_DM, 128, NFF, 128).transpose(2, 1, 0, 3).reshape(NFF, 128, 1024)
    ).astype(bf)
    # w2p: [128, NFF*1024]; [p, cc*1024+n] = W2[cc*128+p, n] (one resident DMA)
    w2p = np.ascontiguousarray(
        W["W2"].reshape(NFF, 128, DM).transpose(1, 0, 2).reshape(128, NFF * 1024)
    ).astype(bf)

    return {
        "xt": xt,
        "wv": wprep_f8(W["Wv"]),
        "wq": wprep_f8(W["Wq"]),
        "st": st.astype(F8NP),
        "wkt": wkt,
        "bqp": bqp,
        # bv folds into the residual: src + bv (probs sum to 1)
        "srcr": np.ascontiguousarray(src_c + W["bv"][None, :]).astype(np.float32),
        "maskp": np.ascontiguousarray(mask_c.reshape(NT, 128).T).astype(np.float32),
        "b1p": np.ascontiguousarray(W["b1"].reshape(NFF, 128).T).astype(np.float32),
        "bb2p": (W["beta1"] + W["b2"]).reshape(1, DM).astype(bf),
        "g1p": W["g1"].reshape(1, DM).astype(bf),
        "g1tp": np.ascontiguousarray(W["g1"].reshape(NC_DM, 128).T).astype(np.float32),
        "be1tp": np.ascontiguousarray(W["beta1"].reshape(NC_DM, 128).T).astype(np.float32),
        "g2p": W["g2"].reshape(1, DM).astype(bf),
        "be2p": W["beta2"].reshape(1, DM).astype(bf),
        "w1p": w1p,
        "w2p": w2p,
    }


def make_in_maps(**inputs):
    inp = {k: np.asarray(v) for k, v in inputs.items()}
    W = {
        k: inp[k]
        for k in ("Wq", "bq", "Wk", "bk", "Wv", "bv", "W1", "b1", "W2", "b2",
                  "g1", "beta1", "g2", "beta2")
    }
    in_maps = []
    for c in range(NCORES):
        sl = slice(c * LC, (c + 1) * LC)
        in_maps.append(_prep_core(inp["src"][sl], inp["target"][sl], inp["attn_mask"][sl], W))
    return in_maps


def get_nc(repeat=1):
    key = ("nc", repeat)
    if key not in _CACHE:
        _CACHE[key] = _build_nc(repeat)
    return _CACHE[key]


def kernel(**inputs) -> np.ndarray:
    nc = get_nc()
    in_maps = make_in_maps(**inputs)
    res = run_bass_kernel_spmd(nc, in_maps, core_ids=list(range(NCORES)))
    return np.concatenate([res.results[c]["out"] for c in range(NCORES)], axis=0)


if __name__ == "__main__":
    import reference

    inputs = {k: np.asarray(v) for k, v in reference.setup_inputs().items()}
    got = kernel(**inputs)
    exp = np.asarray(reference.reference(**inputs))
    err = np.abs(got - exp).max() / np.abs(exp).max()
    print("Relative error:", err)

